# revision 18
# baseline (speedup 1.0000x reference)
"""Trainium2 Bass kernel for a Lorentz (hyperboloid) transformer decoder layer.

Full inputs in, full output out. Data-parallel over batch: core b computes
batch element b (B == 8 == n_cores), no collectives.

Device-side design (per core, one batch element):
  - Activations are kept feature-major [513, 1024] (features on partitions,
    tokens on the free dim), with the ambient order rearranged to
    [512 spatial, 1 time] so 128-row chunks are spatial-aligned.
  - V is computed token-major [tokens, feats] so the attention AV matmul
    needs no transposes anywhere.
  - Attention scores are computed transposed: scoresT[s, t] (keys on
    partitions). Softmax uses two identities: (1) the Minkowski inner
    product of two hyperboloid points is <= -1/K, so scores <= 0 and exp()
    never overflows (no running-max pass); (2) the Lorentz centroid
    projection project(z) = z / sqrt(-<z,z>) is scale-invariant, so the
    softmax denominator cancels and is never computed.
  - Per-token reductions (LayerNorm mean/var, Minkowski norms, time
    recomputation) are partition-dim reductions done on the tensor engine
    as ones/metric matmuls with replicated output rows, so results land
    partition-aligned with their consumers.
  - All matmuls run in fp32r (full PE speed at N=512, ~1.2e-4 rounding).
  - Causal mask via gpsimd affine_select on diagonal key blocks only;
    strictly-masked key blocks are skipped entirely.

Host side: transposes x/enc/weights into device layouts (numpy), splits
per-core inputs, gathers and reassembles the full output.
"""

import numpy as np

import concourse.bass as bass
import concourse.mybir as mybir
import concourse.tile as tile
from concourse import bacc, bass_utils


def _patch_act_tables():
    """Steer every activation to the one func-set that contains all the
    functions this kernel uses (Exp, Ln, Relu, Square, Identity, Copy),
    so at most one LoadActFuncSet is ever needed. Set ids (dict order)
    are preserved; other sets merely stop advertising our functions."""
    import functools
    from concourse import hw_specs
    used = {AF.Exp, AF.Ln, AF.Relu, AF.Square, AF.Identity, AF.Copy,
            AF.Sqrt}
    orig = hw_specs.get_activation_tables
    if getattr(orig, "_lorentz_patched", False):
        return
    raw = orig.__wrapped__

    @functools.cache
    def patched(module_arch):
        tabs = dict(raw(module_arch))
        best = None
        for name, fns in tabs.items():
            if used - {AF.Sqrt} <= fns:
                best = name
                break
        if best is None:
            return tabs
        return {name: (set(fns) if name == best else set(fns) - used)
                for name, fns in tabs.items()}

    patched._lorentz_patched = True
    hw_specs.get_activation_tables = patched
    bacc.get_activation_tables = patched

F32 = mybir.dt.float32
F32R = mybir.dt.float32r
AF = mybir.ActivationFunctionType
ALU = mybir.AluOpType

B, T, S, D, H = 8, 1024, 1024, 513, 8
DS = D - 1            # 512 spatial dims
DH = DS // H          # 64 per-head spatial
DHA = DH + 1          # 65 with time
DFF = 2048
P = 128
NSC = S // P          # 8 key chunks of 128
NTN = 2               # 2 query chunks of 512
TN = 512
NEG = -1e30

COL = {"bq_s": 0, "bk_s": 4, "bq_c": 8, "bk_c": 12, "bo_s": 16,
       "bo_c": 20, "b2": 24}


def _f32(a):
    return np.ascontiguousarray(a, dtype=np.float32)


def _dev_amb(w):
    """[out, 513] weight -> device W^T [513, out], ambient rows reordered
    to [spatial..., time]."""
    return _f32(np.vstack([w[:, 1:].T, w[:, 0:1].T]))


def _make_spec(inputs):
    tgt = np.asarray(inputs["tgt_mask"]).reshape(T, S)
    src = np.asarray(inputs["src_mask"]).reshape(S)
    if tgt.all():
        tgt_kind = "full"
    elif (tgt == np.tril(np.ones((T, S), bool))).all():
        tgt_kind = "causal"
    else:
        tgt_kind = "general"
    src_all = bool(src.all())
    bias_nz = {}
    for n in ["bq_s", "bk_s", "bv_s", "bo_s", "bq_c", "bk_c", "bv_c", "bo_c",
              "b1", "b2"]:
        bias_nz[n] = bool(np.any(np.asarray(inputs[n]) != 0))
    gb_nz = {}
    for i in (1, 2, 3):
        gb_nz[i] = bool(np.any(np.asarray(inputs["g%d" % i]) != 1.0)
                        or np.any(np.asarray(inputs["beta%d" % i]) != 0))
    return (tgt_kind, src_all, tuple(sorted(bias_nz.items())),
            tuple(sorted(gb_nz.items())))


# ----------------------------------------------------------------------------
# device program
# ----------------------------------------------------------------------------

DEBUG_TAPS = False


def _build(spec, repeat=1):
    tgt_kind, src_all, bias_nz_t, gb_nz_t = spec
    bias_nz = dict(bias_nz_t)
    gb_nz = dict(gb_nz_t)

    _patch_act_tables()
    nc = bacc.Bacc("TRN2", debug=False, num_devices=B)

    def din(name, shape, dt=F32R):
        return nc.dram_tensor(name, list(shape), dt, kind="ExternalInput").ap()

    env = {}
    env["x_sp"] = din("x_sp", [DS, T])
    env["x_t"] = din("x_t", [P, T])           # time row, replicated
    env["e_sp"] = din("e_sp", [DS, S])
    for n in ["wq_s", "wk_s", "wv_s", "wq_c", "wk_c", "wv_c"]:
        env[n] = din(n, [DS, DS])
        env[n + "_row"] = din(n + "_row", [1, DS])
    env["wo_s"] = din("wo_s", [DS, DS])
    env["wo_c"] = din("wo_c", [DS, DS])
    env["w1"] = din("w1", [DS, DFF])
    env["w1row"] = din("w1row", [1, DFF])
    env["w2"] = din("w2", [DFF, DS])
    env["rowpack"] = din("rowpack", [1, 1536])
    env["ones_rep"] = din("ones_rep", [P, P])
    env["wmink_rep"] = din("wmink_rep", [DHA, DHA])
    env["wminkcols"] = din("wminkcols", [DHA, 8 * H])
    env["selrep"] = din("selrep", [H, 4 * P])
    env["sel8"] = din("sel8", [P, 32])
    env["epscol"] = din("epscol", [P, 1], F32)
    env["colpack"] = din("colpack", [P, 32], F32)
    if bias_nz["bv_s"] or bias_nz["bv_c"]:
        env["bvrep"] = din("bvrep", [P, 2 * DS], F32)
    if not src_all:
        env["srcb"] = din("srcb", [P, NSC], F32)
    if tgt_kind == "general":
        env["tmaskT"] = din("tmaskT", [S, T])
    if any(gb_nz.values()):
        env["gbcols"] = din("gbcols", [P, 24], F32)
    if bias_nz["b1"]:
        env["b1col"] = din("b1col", [P, 16], F32)
    env["out_sp"] = nc.dram_tensor("out_sp", [DS, T], F32R,
                                   kind="ExternalOutput").ap()
    env["out_t"] = nc.dram_tensor("out_t", [1, T], F32R,
                                  kind="ExternalOutput").ap()
    if DEBUG_TAPS:
        for tap in ["x1", "x2"]:
            env["dbg_" + tap] = nc.dram_tensor(
                "dbg_" + tap, [D, T], F32R, kind="ExternalOutput").ap()
        env["dbg_qasm"] = nc.dram_tensor("dbg_qasm", [DHA, H * TN], F32R,
                                         kind="ExternalOutput").ap()
        env["dbg_kasm"] = nc.dram_tensor("dbg_kasm", [DHA, S], F32R,
                                         kind="ExternalOutput").ap()
        env["dbg_probs"] = nc.dram_tensor("dbg_probs", [P, TN], F32R,
                                          kind="ExternalOutput").ap()
        env["dbg_mid"] = nc.dram_tensor("dbg_mid", [P, TN], F32R,
                                        kind="ExternalOutput").ap()
        env["dbg_vsb"] = nc.dram_tensor("dbg_vsb", [P, H * DHA], F32R,
                                        kind="ExternalOutput").ap()
        env["dbg_probs1"] = nc.dram_tensor("dbg_probs1", [P, TN], F32R,
                                           kind="ExternalOutput").ap()
        env["dbg_midraw"] = nc.dram_tensor("dbg_midraw", [DHA, TN], F32,
                                           kind="ExternalOutput").ap()

    with tile.TileContext(nc) as tc:
        _program(nc, tc, env, tgt_kind, src_all, bias_nz, gb_nz, repeat)
    nc.compile()
    return nc


def _program(nc, tc, env, tgt_kind, src_all, bias_nz, gb_nz, repeat=1):
    from contextlib import ExitStack
    with ExitStack() as ctx:
        shared = ctx.enter_context(tc.tile_pool(name="shared", bufs=1))
        wpool = ctx.enter_context(tc.tile_pool(name="wpool", bufs=1))

        # ---------------- persistent tiles -------------------------------
        xsp = []
        for c in range(4):
            t_ = shared.tile([P, T], F32R, tag=f"xsp{c}", name=f"xsp{c}")
            nc.sync.dma_start(t_[:], env["x_sp"][c * P:(c + 1) * P, :])
            xsp.append(t_)
        xt = shared.tile([P, T], F32R, tag="xt", name="xt")
        nc.sync.dma_start(xt[:], env["x_t"][:])
        esp = []
        for c in range(4):
            t_ = shared.tile([P, S], F32R, tag=f"esp{c}", name=f"esp{c}")
            nc.gpsimd.dma_start(t_[:], env["e_sp"][c * P:(c + 1) * P, :])
            esp.append(t_)
        rowpack = shared.tile([1, 1536], F32R, tag="rowpack", name="rowpack")
        nc.sync.dma_start(rowpack[:], env["rowpack"][:])
        ones_rep = shared.tile([P, P], F32R, tag="ones", name="ones_rep")
        nc.sync.dma_start(ones_rep[:], env["ones_rep"][:])
        wmink = shared.tile([DHA, DHA], F32R, tag="wmink", name="wmink")
        nc.sync.dma_start(wmink[:], env["wmink_rep"][:])
        sel8 = shared.tile([P, 32], F32R, tag="sel8", name="sel8")
        nc.sync.dma_start(sel8[:], env["sel8"][:])
        wminkcols = shared.tile([DHA, 8 * H], F32R, tag="wminkc",
                                name="wminkcols")
        nc.sync.dma_start(wminkcols[:], env["wminkcols"][:])
        selrep = shared.tile([H, 4 * P], F32R, tag="selrep", name="selrep")
        nc.sync.dma_start(selrep[:], env["selrep"][:])
        epscol = shared.tile([P, 1], F32, tag="eps", name="epscol")
        nc.sync.dma_start(epscol[:], env["epscol"][:])
        colpack = shared.tile([P, 32], F32, tag="colpack", name="colpack")
        nc.sync.dma_start(colpack[:], env["colpack"][:])
        bvrep = srcb = None
        if "bvrep" in env:
            bvrep = shared.tile([P, 2 * DS], F32, tag="bvrep", name="bvrep")
            nc.sync.dma_start(bvrep[:], env["bvrep"][:])
        if "srcb" in env:
            srcb = shared.tile([P, NSC], F32, tag="srcb", name="srcb")
            nc.sync.dma_start(srcb[:], env["srcb"][:])
        gbcols = None
        if "gbcols" in env:
            gbcols = shared.tile([P, 24], F32, tag="gbcols", name="gbcols")
            nc.sync.dma_start(gbcols[:], env["gbcols"][:])
        dynrow = shared.tile([P, 1024], F32R, tag="dynrow", name="dynrow")

        ROW_ET = rowpack[0:1, 0:1024]
        ROW_W2 = rowpack[0:1, 1024:1536]

        def load_packed(wtag, dram, row_dram=None):
            """[512, N] weight -> one [P, 4*N (+512)] tile; the K=1 time row
            (if any) is packed at [0:1, 4N:4N+512]."""
            n = dram.shape[1]
            w = wpool.tile([P, 4 * n + (512 if row_dram is not None else 0)],
                           F32R, tag=wtag, name=wtag + "_t")
            nc.sync.dma_start(w[:, 0:4 * n].rearrange("p (c n) -> p c n", c=4),
                              dram.rearrange("(c p) n -> p c n", p=P))
            if row_dram is not None:
                nc.sync.dma_start(w[0:1, 4 * n:4 * n + 512], row_dram[:])
            return w

        # ---------------- residual + projection + LayerNorm --------------
        def resln(pool, pstat_pool, tn, hs, ps_ht, blk):
            """x <- add_time(LN(project(x + 3*h))) in place, slice tn.
            hs: 4 sbuf tiles with h spatial chunks (bias applied).
            ps_ht: psum [P, TN] = sum of h_sp^2 (replicated rows)."""
            sl = slice(tn * TN, (tn + 1) * TN)
            h0 = pool.tile([P, TN], F32, tag="row0", name="h0")
            nc.scalar.activation(h0[:], ps_ht[:], AF.Ln, bias=1.0)
            nc.scalar.activation(h0[:], h0[:], AF.Exp, scale=0.5)
            zt = pool.tile([P, TN], F32, tag="row1", name="zt")
            nc.vector.scalar_tensor_tensor(
                zt[:], in0=h0[:], scalar=3.0, in1=xt[:, sl].bitcast(F32),
                op0=ALU.mult, op1=ALU.add)
            zs = []
            ps_S2 = pstat_pool.tile([P, TN], F32, tag="pstat", bufs=1,
                                    name="psS2")
            for c in range(4):
                z = pool.tile([P, TN], F32R, tag=f"z{c}", name=f"z{c}")
                nc.vector.scalar_tensor_tensor(
                    z[:], in0=hs[c][:], scalar=3.0,
                    in1=xsp[c][:, sl].bitcast(F32), op0=ALU.mult, op1=ALU.add)
                zsq = pool.tile([P, TN], F32R, tag="rsq", bufs=2, name="zsq")
                nc.gpsimd.tensor_mul(zsq[:], z[:], z[:])
                nc.tensor.matmul(ps_S2[:], ones_rep[:], zsq[:],
                                 start=(c == 0), stop=(c == 3))
                zs.append(z)
            A = pool.tile([P, TN], F32, tag="row2", name="A")
            nc.vector.tensor_mul(A[:], zt[:], zt[:])
            nc.vector.tensor_sub(A[:], A[:], ps_S2[:])
            # S2 psum no longer needed after var below reads it; S1 reuses
            # the sbuf-side: compute var pieces from S2 before releasing.
            var = pool.tile([P, TN], F32, tag="row1", name="var")
            nc.vector.tensor_scalar_mul(var[:], ps_S2[:], 1.0 / DS)
            nc.vector.tensor_scalar_max(A[:], A[:], 1e-6)
            nc.scalar.activation(A[:], A[:], AF.Ln)
            nc.scalar.activation(A[:], A[:], AF.Exp, scale=-0.5)
            ps_S1 = pstat_pool.tile([P, TN], F32, tag="pmr", bufs=1,
                                    name="psS1")
            for c in range(4):
                nc.tensor.matmul(ps_S1[:], ones_rep[:], zs[c][:],
                                 start=(c == 0), stop=(c == 3))
            mu = pool.tile([P, TN], F32, tag="row3", name="mu")
            nc.vector.scalar_tensor_tensor(
                mu[:], in0=ps_S1[:], scalar=1.0 / DS, in1=A[:],
                op0=ALU.mult, op1=ALU.mult)
            Asq = pool.tile([P, TN], F32, tag="row0", name="Asq")
            nc.vector.tensor_mul(Asq[:], A[:], A[:])
            nc.vector.tensor_mul(var[:], var[:], Asq[:])
            mu2 = pool.tile([P, TN], F32, tag="row0", name="mu2")
            nc.vector.tensor_mul(mu2[:], mu[:], mu[:])
            nc.vector.tensor_sub(var[:], var[:], mu2[:])
            nc.scalar.activation(var[:], var[:], AF.Ln, bias=epscol[:])
            nc.scalar.activation(var[:], var[:], AF.Exp, scale=-0.5)
            nc.vector.tensor_mul(A[:], A[:], var[:])
            AFr = A
            nc.vector.tensor_mul(mu[:], mu[:], var[:])
            BFr = mu
            ps_yt = pstat_pool.tile([P, TN], F32, tag="pmr", bufs=1,
                                    name="psyt")
            for c in range(4):
                ytmp = pool.tile([P, TN], F32, tag="ytmp", bufs=1, name="ytmp")
                nc.vector.tensor_mul(ytmp[:], zs[c][:].bitcast(F32), AFr[:])
                if gb_nz[blk]:
                    y2 = pool.tile([P, TN], F32, tag="ytmp2", bufs=2,
                                   name="y2")
                    nc.vector.tensor_sub(y2[:], ytmp[:], BFr[:])
                    g_c = gbcols[:, 8 * (blk - 1) + c: 8 * (blk - 1) + c + 1]
                    b_c = gbcols[:, 8 * (blk - 1) + 4 + c:
                                 8 * (blk - 1) + 5 + c]
                    nc.scalar.activation(xsp[c][:, sl], y2[:], AF.Identity,
                                         bias=b_c, scale=g_c)
                else:
                    nc.vector.tensor_sub(xsp[c][:, sl], ytmp[:], BFr[:])
                ysq = pool.tile([P, TN], F32R, tag="rsq", bufs=2, name="ysq")
                nc.gpsimd.tensor_mul(ysq[:], xsp[c][:, sl], xsp[c][:, sl])
                nc.tensor.matmul(ps_yt[:], ones_rep[:], ysq[:],
                                 start=(c == 0), stop=(c == 3))
            nc.scalar.activation(xt[:, sl], ps_yt[:], AF.Ln, bias=1.0)
            nc.scalar.activation(xt[:, sl], xt[:, sl].bitcast(F32), AF.Exp,
                                 scale=0.5)

        # ---------------- attention block --------------------------------
        def attention(apool, psum, kv_sp, kv_t, wq, wk, wv, wo,
                      rq, rk, rv, bcols, blk, causal):
            bq, bk, bv, bo = bcols

            # ---- K projection straight into per-head tiles [65, S] -----
            kh = [apool.tile([DHA, S], F32R, tag=f"kh{h}", name=f"kh{h}")
                  for h in range(H)]
            for tn in range(NTN):
                sl = slice(tn * TN, (tn + 1) * TN)
                ps_kt = psum.tile([H, TN], F32, tag="pstat", bufs=1,
                                  name="pskt")
                for m in range(4):
                    ps_k = psum.tile([P, TN], F32, tag="pp", bufs=2,
                                     name="psk")
                    for kc in range(4):
                        nc.tensor.matmul(
                            ps_k[:],
                            wk[:, kc * DS + m * P: kc * DS + (m + 1) * P],
                            kv_sp[kc][:, sl], start=(kc == 0), stop=False)
                    nc.tensor.matmul(ps_k[:], rk[:, m * P:(m + 1) * P],
                                     kv_t[0:1, sl], start=False, stop=True)
                    ksq = apool.tile([P, TN], F32R, tag="sqs", bufs=2,
                                     name="ksq")
                    for hh in range(2):
                        if bk is not None:
                            nc.vector.tensor_scalar_add(
                                kh[2 * m + hh][0:DH, sl],
                                ps_k[hh * DH:(hh + 1) * DH, :],
                                colpack[:, bk + m: bk + m + 1])
                        else:
                            nc.vector.tensor_copy(
                                kh[2 * m + hh][0:DH, sl],
                                ps_k[hh * DH:(hh + 1) * DH, :])
                    nc.gpsimd.tensor_mul(
                        ksq[0:DH, :], kh[2 * m][0:DH, sl],
                        kh[2 * m][0:DH, sl])
                    nc.gpsimd.tensor_mul(
                        ksq[DH:P, :], kh[2 * m + 1][0:DH, sl],
                        kh[2 * m + 1][0:DH, sl])
                    nc.tensor.matmul(ps_kt[:], sel8[:, m * 8:(m + 1) * 8],
                                     ksq[:], start=(m == 0), stop=(m == 3))
                nc.scalar.activation(dynrow[0:H, sl], ps_kt[:], AF.Ln,
                                     bias=1.0)
                nc.scalar.activation(dynrow[0:H, sl], dynrow[0:H, sl],
                                     AF.Exp, scale=0.5)
                nc.vector.tensor_scalar_mul(dynrow[32:32 + H, sl],
                                            dynrow[0:H, sl], -1.0)
                for h in range(H):
                    nc.sync.dma_start(kh[h][DH:DHA, sl],
                                      dynrow[32 + h:33 + h, sl])

            # ---- V projection (token-major), assembled per head ---------
            vsb = []
            for sc in range(NSC):
                v_ = apool.tile([P, H * DHA], F32R, tag=f"vsb{sc}",
                                name=f"vsb{sc}")
                ps_v = psum.tile([P, TN], F32, tag="pp", bufs=2, name="psv")
                csl = slice(sc * P, (sc + 1) * P)
                for kc in range(4):
                    nc.tensor.matmul(ps_v[:], kv_sp[kc][:, csl],
                                     wv[:, kc * DS:(kc + 1) * DS],
                                     start=(kc == 0), stop=False)
                nc.tensor.matmul(ps_v[:], kv_t[0:1, csl], rv[:],
                                 start=False, stop=True)
                v3 = v_[:].rearrange("p (h c) -> p h c", h=H)
                psv3 = ps_v[:].rearrange("p (h c) -> p h c", h=H)
                if bv is not None:
                    nc.vector.tensor_add(
                        v3[:, :, 0:DH], psv3,
                        bvrep[:, bv:bv + DS].rearrange("p (h c) -> p h c",
                                                       h=H))
                else:
                    nc.vector.tensor_copy(v3[:, :, 0:DH], psv3)
                vsq = apool.tile([P, TN], F32, tag="vsq", bufs=1, name="vsq")
                vsq3 = vsq[:].rearrange("p (h c) -> p h c", h=H)
                nc.gpsimd.tensor_mul(vsq3, v3[:, :, 0:DH], v3[:, :, 0:DH])
                vred = apool.tile([P, H], F32, tag="vred", bufs=1,
                                  name="vred")
                nc.vector.reduce_sum(
                    vred[:], vsq[:].rearrange("p (h c) -> p h c", h=H),
                    axis=mybir.AxisListType.X)
                nc.scalar.activation(vred[:], vred[:], AF.Ln, bias=1.0)
                nc.scalar.activation(v3[:, :, DH:DHA],
                                     vred[:].rearrange("p (h c) -> p h c",
                                                       c=1),
                                     AF.Exp, scale=0.5)
                vsb.append(v_)

            # ---- per query-chunk: Q, scores, AV, project, Wo, LN --------
            for tn in range(NTN):
                sl = slice(tn * TN, (tn + 1) * TN)
                qasm = apool.tile([DHA, H * TN], F32R, tag="qasm",
                                  name="qasm")
                ps_qt = psum.tile([H, TN], F32, tag="pstat", bufs=1,
                                  name="psqt")
                for m in range(4):
                    ps_q = psum.tile([P, TN], F32, tag="pp", bufs=2,
                                     name="psq")
                    for kc in range(4):
                        nc.tensor.matmul(
                            ps_q[:],
                            wq[:, kc * DS + m * P: kc * DS + (m + 1) * P],
                            xsp[kc][:, sl], start=(kc == 0), stop=False)
                    nc.tensor.matmul(ps_q[:], rq[:, m * P:(m + 1) * P],
                                     xt[0:1, sl], start=False, stop=True)
                    qsq = apool.tile([P, TN], F32R, tag="sqs", bufs=2,
                                     name="qsq")
                    for hh in range(2):
                        if bq is not None:
                            nc.vector.tensor_scalar_add(
                                qasm[0:DH, (2 * m + hh) * TN:
                                     (2 * m + hh + 1) * TN],
                                ps_q[hh * DH:(hh + 1) * DH, :],
                                colpack[hh * DH:(hh + 1) * DH,
                                        bq + m: bq + m + 1])
                        else:
                            nc.vector.tensor_copy(
                                qasm[0:DH, (2 * m + hh) * TN:
                                     (2 * m + hh + 1) * TN],
                                ps_q[hh * DH:(hh + 1) * DH, :])
                        nc.vector.tensor_mul(
                            qsq[hh * DH:(hh + 1) * DH, :],
                            qasm[0:DH, (2 * m + hh) * TN:
                                 (2 * m + hh + 1) * TN],
                            qasm[0:DH, (2 * m + hh) * TN:
                                 (2 * m + hh + 1) * TN])
                    nc.tensor.matmul(ps_qt[:], sel8[:, m * 8:(m + 1) * 8],
                                     qsq[:], start=(m == 0), stop=(m == 3))
                nc.scalar.activation(dynrow[64:64 + H, sl], ps_qt[:],
                                     AF.Ln, bias=1.0)
                nc.scalar.activation(dynrow[64:64 + H, sl],
                                     dynrow[64:64 + H, sl], AF.Exp, scale=0.5)
                nc.sync.dma_start(
                    qasm[DH:DHA, :].rearrange("p (h t) -> p h t", h=H),
                    dynrow[64:64 + H, sl])

                nsc = 4 * (tn + 1) if causal else NSC
                mids = [apool.tile([P, TN], F32R, tag=f"mid{mm}",
                                   name=f"mid{mm}") for mm in range(4)]
                ps_nrm = psum.tile([H, TN], F32, tag="pstat", bufs=1,
                                   name="psnrm")
                for h in range(H):
                    kasm = kh[h]
                    probs = []
                    for sc in range(nsc):
                        ps_s = psum.tile([P, TN], F32, tag="pscore", bufs=2,
                                         name="pss")
                        nc.tensor.matmul(ps_s[:],
                                         kasm[:, sc * P:(sc + 1) * P],
                                         qasm[:, h * TN:(h + 1) * TN],
                                         start=True, stop=True)
                        pt = apool.tile([P, TN], F32R, tag="probs", bufs=4,
                                        name="probs")
                        if srcb is not None:
                            nc.scalar.activation(pt[:], ps_s[:], AF.Exp,
                                                 bias=srcb[:, sc:sc + 1],
                                                 scale=0.25)
                        else:
                            nc.scalar.activation(pt[:], ps_s[:], AF.Exp,
                                                 bias=0.0, scale=0.25)
                        if causal and sc >= 4 * tn:
                            # masked entries only exist where t < 128*(r+1)
                            r_ = sc - 4 * tn
                            w_ = min(P * (r_ + 1), TN)
                            nc.gpsimd.affine_select(
                                pt[:, 0:w_], pt[:, 0:w_], pattern=[[1, w_]],
                                compare_op=ALU.is_ge, fill=0.0,
                                base=-P * r_, channel_multiplier=-1)
                        elif tgt_kind == "general":
                            tm = apool.tile([P, TN], F32R, tag="tgtm",
                                            bufs=4, name="tgtm")
                            nc.sync.dma_start(
                                tm[:], env["tmaskT"][sc * P:(sc + 1) * P, sl])
                            pt2 = apool.tile([P, TN], F32R, tag="probs",
                                             bufs=8, name="probs2")
                            nc.vector.tensor_mul(pt2[:], pt[:], tm[:])
                            pt = pt2
                        probs.append(pt)
                    if DEBUG_TAPS and blk == 1 and tn == 0 and h == 0:
                        nc.sync.dma_start(env["dbg_qasm"][:], qasm[:])
                        nc.sync.dma_start(env["dbg_kasm"][:, 0:nsc * P],
                                          kasm[:, 0:nsc * P])
                        nc.sync.dma_start(env["dbg_probs"][:], probs[0][:])
                        nc.sync.dma_start(env["dbg_probs1"][:], probs[1][:])
                        nc.sync.dma_start(env["dbg_vsb"][:], vsb[0][:])
                    ps_m = psum.tile([DHA, TN], F32, tag="pmid", bufs=2,
                                     name="psm")
                    for i in range(nsc):
                        nc.tensor.matmul(ps_m[:],
                                         vsb[i][:, h * DHA:(h + 1) * DHA],
                                         probs[i][:], start=(i == 0),
                                         stop=(i == nsc - 1))
                    msq = apool.tile([DHA, TN], F32R, tag="msq", bufs=1,
                                     name="msq")
                    if DEBUG_TAPS and blk == 1 and tn == 0 and h == 0:
                        mraw = apool.tile([DHA, TN], F32, tag="screp",
                                          bufs=2, name="mraw")
                        nc.vector.tensor_copy(mraw[:], ps_m[:])
                        nc.sync.dma_start(env["dbg_midraw"][:], mraw[:])
                    nc.scalar.square(msq[:], ps_m[:])
                    nc.tensor.matmul(ps_nrm[:],
                                     wminkcols[:, 8 * h:8 * h + 8],
                                     msq[:], start=(h == 0), stop=(h == 7))
                    nc.vector.tensor_copy(
                        mids[h // 2][(h % 2) * DH:(h % 2 + 1) * DH, :],
                        ps_m[0:DH, :])
                # one rsqrt chain for all 8 heads' centroid norms, then
                # broadcast each head's scale row to its 64 partitions
                nrm = apool.tile([H, TN], F32R, tag="nrm", bufs=1, name="nrm")
                nc.vector.tensor_scalar_max(nrm[:], ps_nrm[:], 1e-37)
                nc.scalar.activation(nrm[:], nrm[:].bitcast(F32), AF.Ln)
                nc.scalar.activation(nrm[:], nrm[:].bitcast(F32), AF.Exp,
                                     scale=-0.5)
                for pp_ in range(4):
                    ps_rep = psum.tile([P, TN], F32, tag="pp", bufs=2,
                                       name="psrep")
                    nc.tensor.matmul(ps_rep[:],
                                     selrep[:, pp_ * P:(pp_ + 1) * P],
                                     nrm[:], start=True, stop=True)
                    nc.vector.tensor_mul(mids[pp_][:], mids[pp_][:],
                                         ps_rep[:])

                # ---- Wo projection + residual + LN ----------------------
                hs = []
                ps_ht = psum.tile([P, TN], F32, tag="pmr", bufs=1,
                                  name="psht")
                for mo in range(4):
                    ps_o = psum.tile([P, TN], F32, tag="pp", bufs=2,
                                     name="pso")
                    for kc in range(4):
                        nc.tensor.matmul(
                            ps_o[:],
                            wo[:, kc * DS + mo * P: kc * DS + (mo + 1) * P],
                            mids[kc][:], start=(kc == 0), stop=(kc == 3))
                    hb = apool.tile([P, TN], F32, tag=f"hb{mo}",
                                    name=f"hb{mo}")
                    if bo is not None:
                        nc.vector.tensor_scalar_add(
                            hb[:], ps_o[:], colpack[:, bo + mo: bo + mo + 1])
                    else:
                        nc.vector.tensor_copy(hb[:], ps_o[:])
                    hsq = apool.tile([P, TN], F32R, tag="sqs", bufs=2,
                                     name="hsq")
                    nc.gpsimd.tensor_mul(hsq[:], hb[:], hb[:])
                    nc.tensor.matmul(ps_ht[:], ones_rep[:], hsq[:],
                                     start=(mo == 0), stop=(mo == 3))
                    hs.append(hb)
                resln(apool, psum, tn, hs, ps_ht, blk)
                if DEBUG_TAPS and tn == 0 and blk == 1:
                    nc.sync.dma_start(env["dbg_mid"][:], mids[0][:])

        for rep in range(repeat):
            # ---------------- the three blocks -------------------------------
            with (
                tc.tile_pool(name=f"attn{rep}", bufs=1) as apool,
                tc.tile_pool(name=f"apsum{rep}", bufs=1, space="PSUM") as psum,
            ):
                wq = load_packed("w4", env["wq_s"], env["wq_s_row"])
                wk = load_packed("w5", env["wk_s"], env["wk_s_row"])
                wv = load_packed("w6", env["wv_s"], env["wv_s_row"])
                wo = load_packed("w7", env["wo_s"])
                attention(apool, psum, xsp, xt, wq, wk, wv, wo,
                          wq[0:1, 4 * DS:], wk[0:1, 4 * DS:], wv[0:1, 4 * DS:],
                          (COL["bq_s"] if bias_nz["bq_s"] else None,
                           COL["bk_s"] if bias_nz["bk_s"] else None,
                           0 if bias_nz["bv_s"] else None,
                           COL["bo_s"] if bias_nz["bo_s"] else None),
                          1, tgt_kind == "causal")
                if DEBUG_TAPS:
                    for c in range(4):
                        nc.sync.dma_start(env["dbg_x1"][c * P:(c + 1) * P, :],
                                          xsp[c][:])
                    nc.sync.dma_start(env["dbg_x1"][DS:D, :], xt[0:1, :])
                wq = load_packed("w4", env["wq_c"], env["wq_c_row"])
                wk = load_packed("w5", env["wk_c"], env["wk_c_row"])
                wv = load_packed("w6", env["wv_c"], env["wv_c_row"])
                wo = load_packed("w7", env["wo_c"])
                attention(apool, psum, esp, ROW_ET, wq, wk, wv, wo,
                          wq[0:1, 4 * DS:], wk[0:1, 4 * DS:], wv[0:1, 4 * DS:],
                          (COL["bq_c"] if bias_nz["bq_c"] else None,
                           COL["bk_c"] if bias_nz["bk_c"] else None,
                           DS if bias_nz["bv_c"] else None,
                           COL["bo_c"] if bias_nz["bo_c"] else None),
                          2, False)
                if DEBUG_TAPS:
                    for c in range(4):
                        nc.sync.dma_start(env["dbg_x2"][c * P:(c + 1) * P, :],
                                          xsp[c][:])
                    nc.sync.dma_start(env["dbg_x2"][DS:D, :], xt[0:1, :])

            # ---------------- FFN --------------------------------------------
            with (
                tc.tile_pool(name=f"ffn{rep}", bufs=1) as fpool,
                tc.tile_pool(name=f"fpsum{rep}", bufs=1, space="PSUM") as fpsum,
            ):
                w1t = []
                for c in range(4):
                    w1c = wpool.tile([P, DFF], F32R, tag=f"w{4 + c}",
                                     name=f"w1_{c}")
                    nc.sync.dma_start(w1c[:], env["w1"][c * P:(c + 1) * P, :])
                    w1t.append(w1c)
                w1r = fpool.tile([1, DFF], F32R, tag="w1r", name="w1r")
                nc.sync.dma_start(w1r[:], env["w1row"][:])
                b1c = None
                if bias_nz["b1"]:
                    b1c = fpool.tile([P, 16], F32, tag="b1c", name="b1c")
                    nc.sync.dma_start(b1c[:], env["b1col"][:])
                for tn in range(NTN):
                    sl = slice(tn * TN, (tn + 1) * TN)
                    has = []
                    ps_hat = fpsum.tile([P, TN], F32, tag="pstat", bufs=1,
                                        name="pshat")
                    for f in range(DFF // P):
                        ps_f = fpsum.tile([P, TN], F32, tag="pp", bufs=2,
                                          name="psf")
                        for kc in range(4):
                            nc.tensor.matmul(ps_f[:], w1t[kc][:, f * P:(f + 1) * P],
                                             xsp[kc][:, sl], start=(kc == 0),
                                             stop=False)
                        nc.tensor.matmul(ps_f[:], w1r[0:1, f * P:(f + 1) * P],
                                         xt[0:1, sl], start=False, stop=True)
                        ha = fpool.tile([P, TN], F32R, tag="ha", bufs=6,
                                        name="ha")
                        if b1c is not None:
                            nc.scalar.activation(ha[:], ps_f[:], AF.Relu,
                                                 bias=b1c[:, f:f + 1])
                        else:
                            nc.scalar.activation(ha[:], ps_f[:], AF.Relu)
                        hasq = fpool.tile([P, TN], F32R, tag="sqs", bufs=2,
                                          name="hasq")
                        nc.gpsimd.tensor_mul(hasq[:], ha[:], ha[:])
                        nc.tensor.matmul(ps_hat[:], ones_rep[:], hasq[:],
                                         start=(f == 0), stop=(f == DFF // P - 1))
                        has.append(ha)
                    hat = fpool.tile([P, TN], F32R, tag="hat", bufs=2, name="hat")
                    nc.scalar.activation(hat[:], ps_hat[:], AF.Ln, bias=1.0)
                    nc.scalar.activation(hat[:], hat[:].bitcast(F32), AF.Exp,
                                         scale=0.5)

                    ps_ys = [fpsum.tile([P, TN], F32, tag=f"fy{mo}", bufs=1,
                                        name=f"psy{mo}") for mo in range(4)]
                    for kc in range(DFF // P):
                        w2c = fpool.tile([P, DS], F32R, tag="w2s", bufs=3,
                                         name="w2s")
                        nc.sync.dma_start(w2c[:],
                                          env["w2"][kc * P:(kc + 1) * P, :])
                        for mo in range(4):
                            nc.tensor.matmul(ps_ys[mo][:],
                                             w2c[:, mo * P:(mo + 1) * P],
                                             has[kc][:], start=(kc == 0),
                                             stop=False)
                    for mo in range(4):
                        nc.tensor.matmul(ps_ys[mo][:],
                                         ROW_W2[:, mo * P:(mo + 1) * P],
                                         hat[0:1, :], start=False, stop=True)
                    ps_ft = fpsum.tile([P, TN], F32, tag="pstat", bufs=1,
                                       name="psft")
                    fs = []
                    b2c = COL["b2"] if bias_nz["b2"] else None
                    for mo in range(4):
                        fb = fpool.tile([P, TN], F32, tag=f"hb{mo}",
                                        name=f"fb{mo}")
                        if b2c is not None:
                            nc.vector.tensor_scalar_add(
                                fb[:], ps_ys[mo][:],
                                colpack[:, b2c + mo: b2c + mo + 1])
                        else:
                            nc.vector.tensor_copy(fb[:], ps_ys[mo][:])
                        fsq = fpool.tile([P, TN], F32R, tag="sqs", bufs=2,
                                         name="fsq")
                        nc.gpsimd.tensor_mul(fsq[:], fb[:], fb[:])
                        nc.tensor.matmul(ps_ft[:], ones_rep[:], fsq[:],
                                         start=(mo == 0), stop=(mo == 3))
                        fs.append(fb)
                    resln(fpool, fpsum, tn, fs, ps_ft, 3)

        # ---------------- store output -----------------------------------
        for c in range(4):
            nc.sync.dma_start(env["out_sp"][c * P:(c + 1) * P, :], xsp[c][:])
        nc.sync.dma_start(env["out_t"][:], xt[0:1, :])


# ----------------------------------------------------------------------------
# host wrapper
# ----------------------------------------------------------------------------

_NC_CACHE = {}


def _host_inputs(inputs, spec):
    tgt_kind, src_all, bias_nz_t, gb_nz_t = spec
    bias_nz = dict(bias_nz_t)
    gb_nz = dict(gb_nz_t)

    def g(n):
        return np.asarray(inputs[n])

    shared = {}
    for n, dn in [("Wq_s", "wq_s"), ("Wk_s", "wk_s"), ("Wv_s", "wv_s"),
                  ("Wq_c", "wq_c"), ("Wk_c", "wk_c"), ("Wv_c", "wv_c")]:
        wt = _dev_amb(g(n))
        shared[dn] = _f32(wt[:DS])
        shared[dn + "_row"] = _f32(wt[DS])
    shared["wo_s"] = _f32(g("Wo_s").T)
    shared["wo_c"] = _f32(g("Wo_c").T)
    w1t = _dev_amb(g("W1"))
    shared["w1"] = _f32(w1t[:DS])
    shared["w1row"] = _f32(w1t[DS:DS + 1])
    w2t = _f32(np.vstack([g("W2")[:, 1:].T, g("W2")[:, 0:1].T]))
    shared["w2"] = _f32(w2t[:DFF])

    rowbase = np.zeros((1, 1536), np.float32)
    rowbase[0, 1024:1536] = w2t[DFF]

    ones_rep = np.ones((P, P), np.float32)
    wmink = np.broadcast_to(
        np.concatenate([-np.ones(DH), [1.0]]).astype(np.float32)[:, None],
        (DHA, DHA)).copy()
    minkpat = np.concatenate([-np.ones(DH), [1.0]]).astype(np.float32)
    wminkcols = np.zeros((DHA, 8 * H), np.float32)
    for h in range(H):
        wminkcols[:, 8 * h + h] = minkpat
    selrep = np.zeros((H, 4 * P), np.float32)
    for p_ in range(4):
        for j in range(P):
            selrep[2 * p_ + (1 if j >= DH else 0), p_ * P + j] = 1.0
    sel8 = np.zeros((P, 32), np.float32)
    for m in range(4):
        for pi in range(P):
            sel8[pi, m * 8 + 2 * m + pi // DH] = 1.0
    epscol = np.full((P, 1), 1e-5, np.float32)
    colpack = np.zeros((P, 32), np.float32)
    for n, c0 in COL.items():
        key = {"bq_s": "bq_s", "bk_s": "bk_s", "bq_c": "bq_c",
               "bk_c": "bk_c", "bo_s": "bo_s", "bo_c": "bo_c",
               "b2": "b2"}[n]
        colpack[:, c0:c0 + 4] = g(key).reshape(4, P).T

    per_core_shared = {
        "wq_s": shared["wq_s"], "wk_s": shared["wk_s"],
        "wv_s": shared["wv_s"], "wq_c": shared["wq_c"],
        "wk_c": shared["wk_c"], "wv_c": shared["wv_c"],
        "wo_s": shared["wo_s"], "wo_c": shared["wo_c"],
        "w1": shared["w1"], "w1row": shared["w1row"], "w2": shared["w2"],
        "wq_s_row": shared["wq_s_row"][None, :],
        "wk_s_row": shared["wk_s_row"][None, :],
        "wv_s_row": shared["wv_s_row"][None, :],
        "wq_c_row": shared["wq_c_row"][None, :],
        "wk_c_row": shared["wk_c_row"][None, :],
        "wv_c_row": shared["wv_c_row"][None, :],
        "ones_rep": ones_rep, "wmink_rep": wmink, "sel8": sel8,
        "wminkcols": wminkcols, "selrep": selrep,
        "epscol": epscol, "colpack": colpack,
    }
    if bias_nz["bv_s"] or bias_nz["bv_c"]:
        bvrep = np.zeros((P, 2 * DS), np.float32)
        bvrep[:, 0:DS] = g("bv_s")[None, :]
        bvrep[:, DS:] = g("bv_c")[None, :]
        per_core_shared["bvrep"] = bvrep
    if bias_nz["b1"]:
        per_core_shared["b1col"] = _f32(g("b1").reshape(16, P).T)
    if any(gb_nz.values()):
        gbc = np.zeros((P, 24), np.float32)
        for i in (1, 2, 3):
            gbc[:, 8 * (i - 1):8 * (i - 1) + 4] = g("g%d" % i).reshape(4, P).T
            gbc[:, 8 * (i - 1) + 4:8 * (i - 1) + 8] = \
                g("beta%d" % i).reshape(4, P).T
        per_core_shared["gbcols"] = gbc
    if not src_all:
        srcm = np.asarray(inputs["src_mask"]).reshape(S)
        per_core_shared["srcb"] = _f32(
            np.where(srcm, 0.0, NEG).reshape(NSC, P).T)
    if tgt_kind == "general":
        per_core_shared["tmaskT"] = _f32(
            np.asarray(inputs["tgt_mask"]).reshape(T, S).T)

    x = g("x")
    enc = g("enc_output")
    in_maps = []
    for b in range(B):
        m = dict(per_core_shared)
        m["x_sp"] = _f32(x[b, :, 1:].T)
        m["x_t"] = _f32(np.broadcast_to(x[b, :, 0][None, :], (P, T)))
        m["e_sp"] = _f32(enc[b, :, 1:].T)
        rp = rowbase.copy()
        rp[0, 0:S] = enc[b, :, 0]
        m["rowpack"] = rp
        in_maps.append(m)
    return in_maps


def kernel(**inputs):
    import time as _time
    spec = _make_spec(inputs)
    nc = _NC_CACHE.get(spec)
    if nc is None:
        nc = _build(spec)
        _NC_CACHE[spec] = nc
    in_maps = _host_inputs(inputs, spec)
    res = None
    last_exc = None
    for attempt in range(3):
        try:
            res = bass_utils.run_bass_kernel_spmd(nc, in_maps,
                                                  core_ids=list(range(B)))
            break
        except Exception as e:  # transient device wedge: back off and retry
            last_exc = e
            _time.sleep(5.0)
    if res is None:
        raise last_exc
    out = np.empty((B, T, D), np.float32)
    for b in range(B):
        out[b, :, 1:] = res.results[b]["out_sp"].T
        out[b, :, 0] = res.results[b]["out_t"][0]
    return out



# revision 19
# speedup vs baseline: 1.9620x; 1.9620x over previous
"""Trainium2 Bass kernel for a Lorentz (hyperboloid) transformer decoder layer.

Full inputs in, full output out. Data-parallel over batch: core b computes
batch element b (B == 8 == n_cores), no collectives.

Device-side design (per core, one batch element):
  - Activations are kept feature-major [513, 1024] (features on partitions,
    tokens on the free dim), with the ambient order rearranged to
    [512 spatial, 1 time] so 128-row chunks are spatial-aligned.
  - V is computed token-major [tokens, feats] so the attention AV matmul
    needs no transposes anywhere.
  - Attention scores are computed transposed: scoresT[s, t] (keys on
    partitions). Softmax uses two identities: (1) the Minkowski inner
    product of two hyperboloid points is <= -1/K, so scores <= 0 and exp()
    never overflows (no running-max pass); (2) the Lorentz centroid
    projection project(z) = z / sqrt(-<z,z>) is scale-invariant, so the
    softmax denominator cancels and is never computed.
  - Per-token reductions (LayerNorm mean/var, Minkowski norms, time
    recomputation) are partition-dim reductions done on the tensor engine
    as ones/metric matmuls with replicated output rows, so results land
    partition-aligned with their consumers.
  - All matmuls run in fp32r (full PE speed at N=512, ~1.2e-4 rounding).
  - Causal mask via gpsimd affine_select on diagonal key blocks only;
    strictly-masked key blocks are skipped entirely.

Host side: transposes x/enc/weights into device layouts (numpy), splits
per-core inputs, gathers and reassembles the full output.
"""

import numpy as np

import concourse.bass as bass
import concourse.mybir as mybir
import concourse.tile as tile
from concourse import bacc, bass_utils


def _patch_act_tables():
    """Steer every activation to the one func-set that contains all the
    functions this kernel uses (Exp, Ln, Relu, Square, Identity, Copy),
    so at most one LoadActFuncSet is ever needed. Set ids (dict order)
    are preserved; other sets merely stop advertising our functions."""
    import functools
    from concourse import hw_specs
    used = {AF.Exp, AF.Ln, AF.Relu, AF.Square, AF.Identity, AF.Copy,
            AF.Sqrt}
    orig = hw_specs.get_activation_tables
    if getattr(orig, "_lorentz_patched", False):
        return
    raw = orig.__wrapped__

    @functools.cache
    def patched(module_arch):
        tabs = dict(raw(module_arch))
        best = None
        for name, fns in tabs.items():
            if used - {AF.Sqrt} <= fns:
                best = name
                break
        if best is None:
            return tabs
        return {name: (set(fns) if name == best else set(fns) - used)
                for name, fns in tabs.items()}

    patched._lorentz_patched = True
    hw_specs.get_activation_tables = patched
    bacc.get_activation_tables = patched

F32 = mybir.dt.float32
F32R = mybir.dt.float32r
AF = mybir.ActivationFunctionType
ALU = mybir.AluOpType

B, T, S, D, H = 8, 1024, 1024, 513, 8
DS = D - 1            # 512 spatial dims
DH = DS // H          # 64 per-head spatial
DHA = DH + 1          # 65 with time
DFF = 2048
P = 128
NSC = S // P          # 8 key chunks of 128
NTN = 2               # 2 query chunks of 512
TN = 512
NEG = -1e30

COL = {"bq_s": 0, "bk_s": 4, "bq_c": 8, "bk_c": 12, "bo_s": 16,
       "bo_c": 20, "b2": 24}


def _f32(a):
    return np.ascontiguousarray(a, dtype=np.float32)


def _dev_amb(w):
    """[out, 513] weight -> device W^T [513, out], ambient rows reordered
    to [spatial..., time]."""
    return _f32(np.vstack([w[:, 1:].T, w[:, 0:1].T]))


def _make_spec(inputs):
    tgt = np.asarray(inputs["tgt_mask"]).reshape(T, S)
    src = np.asarray(inputs["src_mask"]).reshape(S)
    if tgt.all():
        tgt_kind = "full"
    elif (tgt == np.tril(np.ones((T, S), bool))).all():
        tgt_kind = "causal"
    else:
        tgt_kind = "general"
    src_all = bool(src.all())
    bias_nz = {}
    for n in ["bq_s", "bk_s", "bv_s", "bo_s", "bq_c", "bk_c", "bv_c", "bo_c",
              "b1", "b2"]:
        bias_nz[n] = bool(np.any(np.asarray(inputs[n]) != 0))
    gb_nz = {}
    for i in (1, 2, 3):
        gb_nz[i] = bool(np.any(np.asarray(inputs["g%d" % i]) != 1.0)
                        or np.any(np.asarray(inputs["beta%d" % i]) != 0))
    return (tgt_kind, src_all, tuple(sorted(bias_nz.items())),
            tuple(sorted(gb_nz.items())))


# ----------------------------------------------------------------------------
# device program
# ----------------------------------------------------------------------------

DEBUG_TAPS = False


def _build(spec, repeat=1):
    tgt_kind, src_all, bias_nz_t, gb_nz_t = spec
    bias_nz = dict(bias_nz_t)
    gb_nz = dict(gb_nz_t)

    _patch_act_tables()
    nc = bacc.Bacc("TRN2", debug=False, num_devices=B)

    def din(name, shape, dt=F32R):
        return nc.dram_tensor(name, list(shape), dt, kind="ExternalInput").ap()

    env = {}
    env["x_sp"] = din("x_sp", [DS, T])
    env["x_t"] = din("x_t", [P, T])           # time row, replicated
    env["e_sp"] = din("e_sp", [DS, S])
    for n in ["wq_s", "wk_s", "wv_s", "wq_c", "wk_c", "wv_c"]:
        env[n] = din(n, [DS, DS])
        env[n + "_row"] = din(n + "_row", [1, DS])
    env["wo_s"] = din("wo_s", [DS, DS])
    env["wo_c"] = din("wo_c", [DS, DS])
    env["w1"] = din("w1", [DS, DFF])
    env["w1row"] = din("w1row", [1, DFF])
    env["w2"] = din("w2", [DFF, DS])
    env["rowpack"] = din("rowpack", [1, 1536])
    env["ones_rep"] = din("ones_rep", [P, P])
    env["wmink_rep"] = din("wmink_rep", [DHA, DHA])
    env["wminkcols"] = din("wminkcols", [DHA, 8 * H])
    env["selrep"] = din("selrep", [H, 4 * P])
    env["sel8"] = din("sel8", [P, 32])
    env["epscol"] = din("epscol", [P, 1], F32)
    env["colpack"] = din("colpack", [P, 32], F32)
    if bias_nz["bv_s"] or bias_nz["bv_c"]:
        env["bvrep"] = din("bvrep", [P, 2 * DS], F32)
    if not src_all:
        env["srcb"] = din("srcb", [P, NSC], F32)
    if tgt_kind == "general":
        env["tmaskT"] = din("tmaskT", [S, T])
    if any(gb_nz.values()):
        env["gbcols"] = din("gbcols", [P, 24], F32)
    if bias_nz["b1"]:
        env["b1col"] = din("b1col", [P, 16], F32)
    env["out_sp"] = nc.dram_tensor("out_sp", [DS, T], F32R,
                                   kind="ExternalOutput").ap()
    env["out_t"] = nc.dram_tensor("out_t", [1, T], F32R,
                                  kind="ExternalOutput").ap()
    if DEBUG_TAPS:
        for tap in ["x1", "x2"]:
            env["dbg_" + tap] = nc.dram_tensor(
                "dbg_" + tap, [D, T], F32R, kind="ExternalOutput").ap()
        env["dbg_qasm"] = nc.dram_tensor("dbg_qasm", [DHA, H * TN], F32R,
                                         kind="ExternalOutput").ap()
        env["dbg_kasm"] = nc.dram_tensor("dbg_kasm", [DHA, S], F32R,
                                         kind="ExternalOutput").ap()
        env["dbg_probs"] = nc.dram_tensor("dbg_probs", [P, TN], F32R,
                                          kind="ExternalOutput").ap()
        env["dbg_mid"] = nc.dram_tensor("dbg_mid", [P, TN], F32R,
                                        kind="ExternalOutput").ap()
        env["dbg_vsb"] = nc.dram_tensor("dbg_vsb", [P, H * DHA], F32R,
                                        kind="ExternalOutput").ap()
        env["dbg_probs1"] = nc.dram_tensor("dbg_probs1", [P, TN], F32R,
                                           kind="ExternalOutput").ap()
        env["dbg_midraw"] = nc.dram_tensor("dbg_midraw", [DHA, TN], F32,
                                           kind="ExternalOutput").ap()

    with tile.TileContext(nc) as tc:
        _program(nc, tc, env, tgt_kind, src_all, bias_nz, gb_nz, repeat)
    nc.compile()
    return nc


def _program(nc, tc, env, tgt_kind, src_all, bias_nz, gb_nz, repeat=1):
    from contextlib import ExitStack
    with ExitStack() as ctx:
        shared = ctx.enter_context(tc.tile_pool(name="shared", bufs=1))
        wpool = ctx.enter_context(tc.tile_pool(name="wpool", bufs=1))

        # ---------------- persistent tiles -------------------------------
        xsp = []
        for c in range(4):
            t_ = shared.tile([P, T], F32R, tag=f"xsp{c}", name=f"xsp{c}")
            nc.sync.dma_start(t_[:], env["x_sp"][c * P:(c + 1) * P, :])
            xsp.append(t_)
        xt = shared.tile([P, T], F32R, tag="xt", name="xt")
        nc.sync.dma_start(xt[:], env["x_t"][:])
        esp = []
        for c in range(4):
            t_ = shared.tile([P, S], F32R, tag=f"esp{c}", name=f"esp{c}")
            nc.gpsimd.dma_start(t_[:], env["e_sp"][c * P:(c + 1) * P, :])
            esp.append(t_)
        rowpack = shared.tile([1, 1536], F32R, tag="rowpack", name="rowpack")
        nc.sync.dma_start(rowpack[:], env["rowpack"][:])
        ones_rep = shared.tile([P, P], F32R, tag="ones", name="ones_rep")
        nc.sync.dma_start(ones_rep[:], env["ones_rep"][:])
        wmink = shared.tile([DHA, DHA], F32R, tag="wmink", name="wmink")
        nc.sync.dma_start(wmink[:], env["wmink_rep"][:])
        sel8 = shared.tile([P, 32], F32R, tag="sel8", name="sel8")
        nc.sync.dma_start(sel8[:], env["sel8"][:])
        wminkcols = shared.tile([DHA, 8 * H], F32R, tag="wminkc",
                                name="wminkcols")
        nc.sync.dma_start(wminkcols[:], env["wminkcols"][:])
        selrep = shared.tile([H, 4 * P], F32R, tag="selrep", name="selrep")
        nc.sync.dma_start(selrep[:], env["selrep"][:])
        epscol = shared.tile([P, 1], F32, tag="eps", name="epscol")
        nc.sync.dma_start(epscol[:], env["epscol"][:])
        colpack = shared.tile([P, 32], F32, tag="colpack", name="colpack")
        nc.sync.dma_start(colpack[:], env["colpack"][:])
        bvrep = srcb = None
        if "bvrep" in env:
            bvrep = shared.tile([P, 2 * DS], F32, tag="bvrep", name="bvrep")
            nc.sync.dma_start(bvrep[:], env["bvrep"][:])
        if "srcb" in env:
            srcb = shared.tile([P, NSC], F32, tag="srcb", name="srcb")
            nc.sync.dma_start(srcb[:], env["srcb"][:])
        gbcols = None
        if "gbcols" in env:
            gbcols = shared.tile([P, 24], F32, tag="gbcols", name="gbcols")
            nc.sync.dma_start(gbcols[:], env["gbcols"][:])
        dynrow = shared.tile([P, 1024], F32R, tag="dynrow", name="dynrow")

        ROW_ET = rowpack[0:1, 0:1024]
        ROW_W2 = rowpack[0:1, 1024:1536]

        def load_packed(wtag, dram, row_dram=None):
            """[512, N] weight -> one [P, 4*N (+512)] tile; the K=1 time row
            (if any) is packed at [0:1, 4N:4N+512]."""
            n = dram.shape[1]
            w = wpool.tile([P, 4 * n + (512 if row_dram is not None else 0)],
                           F32R, tag=wtag, name=wtag + "_t")
            nc.sync.dma_start(w[:, 0:4 * n].rearrange("p (c n) -> p c n", c=4),
                              dram.rearrange("(c p) n -> p c n", p=P))
            if row_dram is not None:
                nc.sync.dma_start(w[0:1, 4 * n:4 * n + 512], row_dram[:])
            return w

        # ---------------- residual + projection + LayerNorm --------------
        def resln(pool, pstat_pool, tn, hs, ps_ht, blk):
            """x <- add_time(LN(project(x + 3*h))) in place, slice tn.
            hs: 4 sbuf tiles with h spatial chunks (bias applied).
            ps_ht: psum [P, TN] = sum of h_sp^2 (replicated rows)."""
            sl = slice(tn * TN, (tn + 1) * TN)
            h0 = pool.tile([P, TN], F32, tag="row0", name="h0")
            nc.scalar.activation(h0[:], ps_ht[:], AF.Ln, bias=1.0)
            nc.scalar.activation(h0[:], h0[:], AF.Exp, scale=0.5)
            zt = pool.tile([P, TN], F32, tag="row1", name="zt")
            nc.vector.scalar_tensor_tensor(
                zt[:], in0=h0[:], scalar=3.0, in1=xt[:, sl].bitcast(F32),
                op0=ALU.mult, op1=ALU.add)
            zs = []
            ps_S2 = pstat_pool.tile([P, TN], F32, tag="pstat", bufs=1,
                                    name="psS2")
            for c in range(4):
                z = pool.tile([P, TN], F32R, tag=f"z{c}", name=f"z{c}")
                nc.vector.scalar_tensor_tensor(
                    z[:], in0=hs[c][:], scalar=3.0,
                    in1=xsp[c][:, sl].bitcast(F32), op0=ALU.mult, op1=ALU.add)
                zsq = pool.tile([P, TN], F32R, tag="rsq", bufs=2, name="zsq")
                nc.gpsimd.tensor_mul(zsq[:], z[:], z[:])
                nc.tensor.matmul(ps_S2[:], ones_rep[:], zsq[:],
                                 start=(c == 0), stop=(c == 3))
                zs.append(z)
            A = pool.tile([P, TN], F32, tag="row2", name="A")
            nc.vector.tensor_mul(A[:], zt[:], zt[:])
            nc.vector.tensor_sub(A[:], A[:], ps_S2[:])
            # S2 psum no longer needed after var below reads it; S1 reuses
            # the sbuf-side: compute var pieces from S2 before releasing.
            var = pool.tile([P, TN], F32, tag="row1", name="var")
            nc.vector.tensor_scalar_mul(var[:], ps_S2[:], 1.0 / DS)
            nc.vector.tensor_scalar_max(A[:], A[:], 1e-6)
            nc.scalar.activation(A[:], A[:], AF.Ln)
            nc.scalar.activation(A[:], A[:], AF.Exp, scale=-0.5)
            ps_S1 = pstat_pool.tile([P, TN], F32, tag="pstat", bufs=1,
                                    name="psS1")
            for c in range(4):
                nc.tensor.matmul(ps_S1[:], ones_rep[:], zs[c][:],
                                 start=(c == 0), stop=(c == 3))
            mu = pool.tile([P, TN], F32, tag="row3", name="mu")
            nc.vector.scalar_tensor_tensor(
                mu[:], in0=ps_S1[:], scalar=1.0 / DS, in1=A[:],
                op0=ALU.mult, op1=ALU.mult)
            Asq = pool.tile([P, TN], F32, tag="row0", name="Asq")
            nc.vector.tensor_mul(Asq[:], A[:], A[:])
            nc.vector.tensor_mul(var[:], var[:], Asq[:])
            mu2 = pool.tile([P, TN], F32, tag="row0", name="mu2")
            nc.vector.tensor_mul(mu2[:], mu[:], mu[:])
            nc.vector.tensor_sub(var[:], var[:], mu2[:])
            nc.scalar.activation(var[:], var[:], AF.Ln, bias=epscol[:])
            nc.scalar.activation(var[:], var[:], AF.Exp, scale=-0.5)
            nc.vector.tensor_mul(A[:], A[:], var[:])
            AFr = A
            nc.vector.tensor_mul(mu[:], mu[:], var[:])
            BFr = mu
            ps_yt = pstat_pool.tile([P, TN], F32, tag="pmr", bufs=1,
                                    name="psyt")
            for c in range(4):
                ytmp = pool.tile([P, TN], F32, tag="ytmp", bufs=1, name="ytmp")
                nc.vector.tensor_mul(ytmp[:], zs[c][:].bitcast(F32), AFr[:])
                if gb_nz[blk]:
                    y2 = pool.tile([P, TN], F32, tag="ytmp2", bufs=2,
                                   name="y2")
                    nc.vector.tensor_sub(y2[:], ytmp[:], BFr[:])
                    g_c = gbcols[:, 8 * (blk - 1) + c: 8 * (blk - 1) + c + 1]
                    b_c = gbcols[:, 8 * (blk - 1) + 4 + c:
                                 8 * (blk - 1) + 5 + c]
                    nc.scalar.activation(xsp[c][:, sl], y2[:], AF.Identity,
                                         bias=b_c, scale=g_c)
                else:
                    nc.vector.tensor_sub(xsp[c][:, sl], ytmp[:], BFr[:])
                ysq = pool.tile([P, TN], F32R, tag="rsq", bufs=2, name="ysq")
                nc.gpsimd.tensor_mul(ysq[:], xsp[c][:, sl], xsp[c][:, sl])
                nc.tensor.matmul(ps_yt[:], ones_rep[:], ysq[:],
                                 start=(c == 0), stop=(c == 3))
            nc.scalar.activation(xt[:, sl], ps_yt[:], AF.Ln, bias=1.0)
            nc.scalar.activation(xt[:, sl], xt[:, sl].bitcast(F32), AF.Exp,
                                 scale=0.5)

        # ---------------- attention block --------------------------------
        def attention(apool, psum, kv_sp, kv_t, wq, wk, wv, wo,
                      rq, rk, rv, bcols, blk, causal):
            bq, bk, bv, bo = bcols

            # ---- K projection straight into per-head tiles [65, S] -----
            kh = [apool.tile([DHA, S], F32R, tag=f"kh{h}", name=f"kh{h}")
                  for h in range(H)]
            for tn in range(NTN):
                sl = slice(tn * TN, (tn + 1) * TN)
                ps_kt = psum.tile([H, TN], F32, tag="pstat", bufs=1,
                                  name="pskt")
                for m in range(4):
                    ps_k = psum.tile([P, TN], F32, tag="pp", bufs=2,
                                     name="psk")
                    for kc in range(4):
                        nc.tensor.matmul(
                            ps_k[:],
                            wk[:, kc * DS + m * P: kc * DS + (m + 1) * P],
                            kv_sp[kc][:, sl], start=(kc == 0), stop=False)
                    nc.tensor.matmul(ps_k[:], rk[:, m * P:(m + 1) * P],
                                     kv_t[0:1, sl], start=False, stop=True)
                    ksq = apool.tile([P, TN], F32R, tag="sqs", bufs=2,
                                     name="ksq")
                    for hh in range(2):
                        if bk is not None:
                            nc.vector.tensor_scalar_add(
                                kh[2 * m + hh][0:DH, sl],
                                ps_k[hh * DH:(hh + 1) * DH, :],
                                colpack[:, bk + m: bk + m + 1])
                        else:
                            nc.vector.tensor_copy(
                                kh[2 * m + hh][0:DH, sl],
                                ps_k[hh * DH:(hh + 1) * DH, :])
                    nc.gpsimd.tensor_mul(
                        ksq[0:DH, :], kh[2 * m][0:DH, sl],
                        kh[2 * m][0:DH, sl])
                    nc.gpsimd.tensor_mul(
                        ksq[DH:P, :], kh[2 * m + 1][0:DH, sl],
                        kh[2 * m + 1][0:DH, sl])
                    nc.tensor.matmul(ps_kt[:], sel8[:, m * 8:(m + 1) * 8],
                                     ksq[:], start=(m == 0), stop=(m == 3))
                nc.scalar.activation(dynrow[0:H, sl], ps_kt[:], AF.Ln,
                                     bias=1.0)
                nc.scalar.activation(dynrow[0:H, sl], dynrow[0:H, sl],
                                     AF.Exp, scale=0.5)
                nc.vector.tensor_scalar_mul(dynrow[32:32 + H, sl],
                                            dynrow[0:H, sl], -1.0)
                for h in range(H):
                    nc.sync.dma_start(kh[h][DH:DHA, sl],
                                      dynrow[32 + h:33 + h, sl])

            # ---- V projection (token-major), assembled per head ---------
            vsb = []
            for sc in range(NSC):
                v_ = apool.tile([P, H * DHA], F32R, tag=f"vsb{sc}",
                                name=f"vsb{sc}")
                ps_v = psum.tile([P, TN], F32, tag="pp", bufs=2, name="psv")
                csl = slice(sc * P, (sc + 1) * P)
                for kc in range(4):
                    nc.tensor.matmul(ps_v[:], kv_sp[kc][:, csl],
                                     wv[:, kc * DS:(kc + 1) * DS],
                                     start=(kc == 0), stop=False)
                nc.tensor.matmul(ps_v[:], kv_t[0:1, csl], rv[:],
                                 start=False, stop=True)
                v3 = v_[:].rearrange("p (h c) -> p h c", h=H)
                psv3 = ps_v[:].rearrange("p (h c) -> p h c", h=H)
                if bv is not None:
                    nc.vector.tensor_add(
                        v3[:, :, 0:DH], psv3,
                        bvrep[:, bv:bv + DS].rearrange("p (h c) -> p h c",
                                                       h=H))
                else:
                    nc.vector.tensor_copy(v3[:, :, 0:DH], psv3)
                vsq = apool.tile([P, TN], F32, tag="vsq", bufs=1, name="vsq")
                vsq3 = vsq[:].rearrange("p (h c) -> p h c", h=H)
                nc.gpsimd.tensor_mul(vsq3, v3[:, :, 0:DH], v3[:, :, 0:DH])
                vred = apool.tile([P, H], F32, tag="vred", bufs=1,
                                  name="vred")
                nc.vector.reduce_sum(
                    vred[:], vsq[:].rearrange("p (h c) -> p h c", h=H),
                    axis=mybir.AxisListType.X)
                nc.scalar.activation(vred[:], vred[:], AF.Ln, bias=1.0)
                nc.scalar.activation(v3[:, :, DH:DHA],
                                     vred[:].rearrange("p (h c) -> p h c",
                                                       c=1),
                                     AF.Exp, scale=0.5)
                vsb.append(v_)

            # ---- per query-chunk: Q, scores, AV, project, Wo, LN --------
            for tn in range(NTN):
                sl = slice(tn * TN, (tn + 1) * TN)
                qasm = apool.tile([DHA, H * TN], F32R, tag="qasm",
                                  name="qasm")
                ps_qt = psum.tile([H, TN], F32, tag="pstat", bufs=1,
                                  name="psqt")
                for m in range(4):
                    ps_q = psum.tile([P, TN], F32, tag="pp", bufs=2,
                                     name="psq")
                    for kc in range(4):
                        nc.tensor.matmul(
                            ps_q[:],
                            wq[:, kc * DS + m * P: kc * DS + (m + 1) * P],
                            xsp[kc][:, sl], start=(kc == 0), stop=False)
                    nc.tensor.matmul(ps_q[:], rq[:, m * P:(m + 1) * P],
                                     xt[0:1, sl], start=False, stop=True)
                    qsq = apool.tile([P, TN], F32R, tag="sqs", bufs=2,
                                     name="qsq")
                    for hh in range(2):
                        if bq is not None:
                            nc.vector.tensor_scalar_add(
                                qasm[0:DH, (2 * m + hh) * TN:
                                     (2 * m + hh + 1) * TN],
                                ps_q[hh * DH:(hh + 1) * DH, :],
                                colpack[hh * DH:(hh + 1) * DH,
                                        bq + m: bq + m + 1])
                        else:
                            nc.vector.tensor_copy(
                                qasm[0:DH, (2 * m + hh) * TN:
                                     (2 * m + hh + 1) * TN],
                                ps_q[hh * DH:(hh + 1) * DH, :])
                        nc.vector.tensor_mul(
                            qsq[hh * DH:(hh + 1) * DH, :],
                            qasm[0:DH, (2 * m + hh) * TN:
                                 (2 * m + hh + 1) * TN],
                            qasm[0:DH, (2 * m + hh) * TN:
                                 (2 * m + hh + 1) * TN])
                    nc.tensor.matmul(ps_qt[:], sel8[:, m * 8:(m + 1) * 8],
                                     qsq[:], start=(m == 0), stop=(m == 3))
                nc.scalar.activation(dynrow[64:64 + H, sl], ps_qt[:],
                                     AF.Ln, bias=1.0)
                nc.scalar.activation(dynrow[64:64 + H, sl],
                                     dynrow[64:64 + H, sl], AF.Exp, scale=0.5)
                nc.sync.dma_start(
                    qasm[DH:DHA, :].rearrange("p (h t) -> p h t", h=H),
                    dynrow[64:64 + H, sl])

                nsc = 4 * (tn + 1) if causal else NSC
                mids = [apool.tile([P, TN], F32R, tag=f"mid{mm}",
                                   name=f"mid{mm}") for mm in range(4)]
                ps_nrm = psum.tile([H, TN], F32, tag="pstat", bufs=1,
                                   name="psnrm")
                for h in range(H):
                    kasm = kh[h]
                    probs = []
                    for sc in range(nsc):
                        ps_s = psum.tile([P, TN], F32, tag="pscore", bufs=2,
                                         name="pss")
                        nc.tensor.matmul(ps_s[:],
                                         kasm[:, sc * P:(sc + 1) * P],
                                         qasm[:, h * TN:(h + 1) * TN],
                                         start=True, stop=True)
                        pt = apool.tile([P, TN], F32R, tag="probs", bufs=4,
                                        name="probs")
                        if srcb is not None:
                            nc.scalar.activation(pt[:], ps_s[:], AF.Exp,
                                                 bias=srcb[:, sc:sc + 1],
                                                 scale=0.25)
                        else:
                            nc.scalar.activation(pt[:], ps_s[:], AF.Exp,
                                                 bias=0.0, scale=0.25)
                        if causal and sc >= 4 * tn:
                            # masked entries only exist where t < 128*(r+1)
                            r_ = sc - 4 * tn
                            w_ = min(P * (r_ + 1), TN)
                            nc.gpsimd.affine_select(
                                pt[:, 0:w_], pt[:, 0:w_], pattern=[[1, w_]],
                                compare_op=ALU.is_ge, fill=0.0,
                                base=-P * r_, channel_multiplier=-1)
                        elif tgt_kind == "general":
                            tm = apool.tile([P, TN], F32R, tag="tgtm",
                                            bufs=4, name="tgtm")
                            nc.sync.dma_start(
                                tm[:], env["tmaskT"][sc * P:(sc + 1) * P, sl])
                            pt2 = apool.tile([P, TN], F32R, tag="probs",
                                             bufs=8, name="probs2")
                            nc.vector.tensor_mul(pt2[:], pt[:], tm[:])
                            pt = pt2
                        probs.append(pt)
                    if DEBUG_TAPS and blk == 1 and tn == 0 and h == 0:
                        nc.sync.dma_start(env["dbg_qasm"][:], qasm[:])
                        nc.sync.dma_start(env["dbg_kasm"][:, 0:nsc * P],
                                          kasm[:, 0:nsc * P])
                        nc.sync.dma_start(env["dbg_probs"][:], probs[0][:])
                        nc.sync.dma_start(env["dbg_probs1"][:], probs[1][:])
                        nc.sync.dma_start(env["dbg_vsb"][:], vsb[0][:])
                    ps_m = psum.tile([DHA, TN], F32, tag="pmid", bufs=2,
                                     name="psm")
                    for i in range(nsc):
                        nc.tensor.matmul(ps_m[:],
                                         vsb[i][:, h * DHA:(h + 1) * DHA],
                                         probs[i][:], start=(i == 0),
                                         stop=(i == nsc - 1))
                    msq = apool.tile([DHA, TN], F32R, tag="msq", bufs=1,
                                     name="msq")
                    if DEBUG_TAPS and blk == 1 and tn == 0 and h == 0:
                        mraw = apool.tile([DHA, TN], F32, tag="screp",
                                          bufs=2, name="mraw")
                        nc.vector.tensor_copy(mraw[:], ps_m[:])
                        nc.sync.dma_start(env["dbg_midraw"][:], mraw[:])
                    nc.scalar.square(msq[:], ps_m[:])
                    nc.tensor.matmul(ps_nrm[:],
                                     wminkcols[:, 8 * h:8 * h + 8],
                                     msq[:], start=(h == 0), stop=(h == 7))
                    nc.vector.tensor_copy(
                        mids[h // 2][(h % 2) * DH:(h % 2 + 1) * DH, :],
                        ps_m[0:DH, :])
                # one rsqrt chain for all 8 heads' centroid norms, then
                # broadcast each head's scale row to its 64 partitions
                nrm = apool.tile([H, TN], F32R, tag="nrm", bufs=1, name="nrm")
                nc.vector.tensor_scalar_max(nrm[:], ps_nrm[:], 1e-37)
                nc.scalar.activation(nrm[:], nrm[:].bitcast(F32), AF.Ln)
                nc.scalar.activation(nrm[:], nrm[:].bitcast(F32), AF.Exp,
                                     scale=-0.5)
                for pp_ in range(4):
                    ps_rep = psum.tile([P, TN], F32, tag="pp", bufs=2,
                                       name="psrep")
                    nc.tensor.matmul(ps_rep[:],
                                     selrep[:, pp_ * P:(pp_ + 1) * P],
                                     nrm[:], start=True, stop=True)
                    nc.vector.tensor_mul(mids[pp_][:], mids[pp_][:],
                                         ps_rep[:])

                # ---- Wo projection + residual + LN ----------------------
                hs = []
                ps_ht = psum.tile([P, TN], F32, tag="pmr", bufs=1,
                                  name="psht")
                for mo in range(4):
                    ps_o = psum.tile([P, TN], F32, tag="pp", bufs=2,
                                     name="pso")
                    for kc in range(4):
                        nc.tensor.matmul(
                            ps_o[:],
                            wo[:, kc * DS + mo * P: kc * DS + (mo + 1) * P],
                            mids[kc][:], start=(kc == 0), stop=(kc == 3))
                    hb = apool.tile([P, TN], F32, tag=f"hb{mo}",
                                    name=f"hb{mo}")
                    if bo is not None:
                        nc.vector.tensor_scalar_add(
                            hb[:], ps_o[:], colpack[:, bo + mo: bo + mo + 1])
                    else:
                        nc.vector.tensor_copy(hb[:], ps_o[:])
                    hsq = apool.tile([P, TN], F32R, tag="sqs", bufs=2,
                                     name="hsq")
                    nc.gpsimd.tensor_mul(hsq[:], hb[:], hb[:])
                    nc.tensor.matmul(ps_ht[:], ones_rep[:], hsq[:],
                                     start=(mo == 0), stop=(mo == 3))
                    hs.append(hb)
                resln(apool, psum, tn, hs, ps_ht, blk)
                if DEBUG_TAPS and tn == 0 and blk == 1:
                    nc.sync.dma_start(env["dbg_mid"][:], mids[0][:])

        for rep in range(repeat):
            # ---------------- the three blocks -------------------------------
            with (
                tc.tile_pool(name=f"attn{rep}", bufs=1) as apool,
                tc.tile_pool(name=f"apsum{rep}", bufs=1, space="PSUM") as psum,
            ):
                wq = load_packed("w4", env["wq_s"], env["wq_s_row"])
                wk = load_packed("w5", env["wk_s"], env["wk_s_row"])
                wv = load_packed("w6", env["wv_s"], env["wv_s_row"])
                wo = load_packed("w7", env["wo_s"])
                attention(apool, psum, xsp, xt, wq, wk, wv, wo,
                          wq[0:1, 4 * DS:], wk[0:1, 4 * DS:], wv[0:1, 4 * DS:],
                          (COL["bq_s"] if bias_nz["bq_s"] else None,
                           COL["bk_s"] if bias_nz["bk_s"] else None,
                           0 if bias_nz["bv_s"] else None,
                           COL["bo_s"] if bias_nz["bo_s"] else None),
                          1, tgt_kind == "causal")
                if DEBUG_TAPS:
                    for c in range(4):
                        nc.sync.dma_start(env["dbg_x1"][c * P:(c + 1) * P, :],
                                          xsp[c][:])
                    nc.sync.dma_start(env["dbg_x1"][DS:D, :], xt[0:1, :])
                wq = load_packed("w4", env["wq_c"], env["wq_c_row"])
                wk = load_packed("w5", env["wk_c"], env["wk_c_row"])
                wv = load_packed("w6", env["wv_c"], env["wv_c_row"])
                wo = load_packed("w7", env["wo_c"])
                attention(apool, psum, esp, ROW_ET, wq, wk, wv, wo,
                          wq[0:1, 4 * DS:], wk[0:1, 4 * DS:], wv[0:1, 4 * DS:],
                          (COL["bq_c"] if bias_nz["bq_c"] else None,
                           COL["bk_c"] if bias_nz["bk_c"] else None,
                           DS if bias_nz["bv_c"] else None,
                           COL["bo_c"] if bias_nz["bo_c"] else None),
                          2, False)
                if DEBUG_TAPS:
                    for c in range(4):
                        nc.sync.dma_start(env["dbg_x2"][c * P:(c + 1) * P, :],
                                          xsp[c][:])
                    nc.sync.dma_start(env["dbg_x2"][DS:D, :], xt[0:1, :])

            # ---------------- FFN --------------------------------------------
            with (
                tc.tile_pool(name=f"ffn{rep}", bufs=1) as fpool,
                tc.tile_pool(name=f"fpsum{rep}", bufs=1, space="PSUM") as fpsum,
            ):
                w1t = []
                for c in range(4):
                    w1c = wpool.tile([P, DFF], F32R, tag=f"w{4 + c}",
                                     name=f"w1_{c}")
                    nc.sync.dma_start(w1c[:], env["w1"][c * P:(c + 1) * P, :])
                    w1t.append(w1c)
                w1r = fpool.tile([1, DFF], F32R, tag="w1r", name="w1r")
                nc.sync.dma_start(w1r[:], env["w1row"][:])
                b1c = None
                if bias_nz["b1"]:
                    b1c = fpool.tile([P, 16], F32, tag="b1c", name="b1c")
                    nc.sync.dma_start(b1c[:], env["b1col"][:])
                for tn in range(NTN):
                    sl = slice(tn * TN, (tn + 1) * TN)
                    has = []
                    ps_hat = fpsum.tile([P, TN], F32, tag="pstat", bufs=1,
                                        name="pshat")
                    for f in range(DFF // P):
                        ps_f = fpsum.tile([P, TN], F32, tag="pp", bufs=2,
                                          name="psf")
                        for kc in range(4):
                            nc.tensor.matmul(ps_f[:], w1t[kc][:, f * P:(f + 1) * P],
                                             xsp[kc][:, sl], start=(kc == 0),
                                             stop=False)
                        nc.tensor.matmul(ps_f[:], w1r[0:1, f * P:(f + 1) * P],
                                         xt[0:1, sl], start=False, stop=True)
                        ha = fpool.tile([P, TN], F32R, tag="ha", bufs=6,
                                        name="ha")
                        if b1c is not None:
                            nc.scalar.activation(ha[:], ps_f[:], AF.Relu,
                                                 bias=b1c[:, f:f + 1])
                        else:
                            nc.scalar.activation(ha[:], ps_f[:], AF.Relu)
                        hasq = fpool.tile([P, TN], F32R, tag="sqs", bufs=2,
                                          name="hasq")
                        nc.gpsimd.tensor_mul(hasq[:], ha[:], ha[:])
                        nc.tensor.matmul(ps_hat[:], ones_rep[:], hasq[:],
                                         start=(f == 0), stop=(f == DFF // P - 1))
                        has.append(ha)
                    hat = fpool.tile([P, TN], F32R, tag="hat", bufs=2, name="hat")
                    nc.scalar.activation(hat[:], ps_hat[:], AF.Ln, bias=1.0)
                    nc.scalar.activation(hat[:], hat[:].bitcast(F32), AF.Exp,
                                         scale=0.5)

                    ps_ys = [fpsum.tile([P, TN], F32, tag=f"fy{mo}", bufs=1,
                                        name=f"psy{mo}") for mo in range(4)]
                    for kc in range(DFF // P):
                        w2c = fpool.tile([P, DS], F32R, tag="w2s", bufs=3,
                                         name="w2s")
                        nc.sync.dma_start(w2c[:],
                                          env["w2"][kc * P:(kc + 1) * P, :])
                        for mo in range(4):
                            nc.tensor.matmul(ps_ys[mo][:],
                                             w2c[:, mo * P:(mo + 1) * P],
                                             has[kc][:], start=(kc == 0),
                                             stop=False)
                    for mo in range(4):
                        nc.tensor.matmul(ps_ys[mo][:],
                                         ROW_W2[:, mo * P:(mo + 1) * P],
                                         hat[0:1, :], start=False, stop=True)
                    ps_ft = fpsum.tile([P, TN], F32, tag="pstat", bufs=1,
                                       name="psft")
                    fs = []
                    b2c = COL["b2"] if bias_nz["b2"] else None
                    for mo in range(4):
                        fb = fpool.tile([P, TN], F32, tag=f"hb{mo}",
                                        name=f"fb{mo}")
                        if b2c is not None:
                            nc.vector.tensor_scalar_add(
                                fb[:], ps_ys[mo][:],
                                colpack[:, b2c + mo: b2c + mo + 1])
                        else:
                            nc.vector.tensor_copy(fb[:], ps_ys[mo][:])
                        fsq = fpool.tile([P, TN], F32R, tag="sqs", bufs=2,
                                         name="fsq")
                        nc.gpsimd.tensor_mul(fsq[:], fb[:], fb[:])
                        nc.tensor.matmul(ps_ft[:], ones_rep[:], fsq[:],
                                         start=(mo == 0), stop=(mo == 3))
                        fs.append(fb)
                    resln(fpool, fpsum, tn, fs, ps_ft, 3)

        # ---------------- store output -----------------------------------
        for c in range(4):
            nc.sync.dma_start(env["out_sp"][c * P:(c + 1) * P, :], xsp[c][:])
        nc.sync.dma_start(env["out_t"][:], xt[0:1, :])


# ----------------------------------------------------------------------------
# host wrapper
# ----------------------------------------------------------------------------

_NC_CACHE = {}


def _host_inputs(inputs, spec):
    tgt_kind, src_all, bias_nz_t, gb_nz_t = spec
    bias_nz = dict(bias_nz_t)
    gb_nz = dict(gb_nz_t)

    def g(n):
        return np.asarray(inputs[n])

    shared = {}
    for n, dn in [("Wq_s", "wq_s"), ("Wk_s", "wk_s"), ("Wv_s", "wv_s"),
                  ("Wq_c", "wq_c"), ("Wk_c", "wk_c"), ("Wv_c", "wv_c")]:
        wt = _dev_amb(g(n))
        shared[dn] = _f32(wt[:DS])
        shared[dn + "_row"] = _f32(wt[DS])
    shared["wo_s"] = _f32(g("Wo_s").T)
    shared["wo_c"] = _f32(g("Wo_c").T)
    w1t = _dev_amb(g("W1"))
    shared["w1"] = _f32(w1t[:DS])
    shared["w1row"] = _f32(w1t[DS:DS + 1])
    w2t = _f32(np.vstack([g("W2")[:, 1:].T, g("W2")[:, 0:1].T]))
    shared["w2"] = _f32(w2t[:DFF])

    rowbase = np.zeros((1, 1536), np.float32)
    rowbase[0, 1024:1536] = w2t[DFF]

    ones_rep = np.ones((P, P), np.float32)
    wmink = np.broadcast_to(
        np.concatenate([-np.ones(DH), [1.0]]).astype(np.float32)[:, None],
        (DHA, DHA)).copy()
    minkpat = np.concatenate([-np.ones(DH), [1.0]]).astype(np.float32)
    wminkcols = np.zeros((DHA, 8 * H), np.float32)
    for h in range(H):
        wminkcols[:, 8 * h + h] = minkpat
    selrep = np.zeros((H, 4 * P), np.float32)
    for p_ in range(4):
        for j in range(P):
            selrep[2 * p_ + (1 if j >= DH else 0), p_ * P + j] = 1.0
    sel8 = np.zeros((P, 32), np.float32)
    for m in range(4):
        for pi in range(P):
            sel8[pi, m * 8 + 2 * m + pi // DH] = 1.0
    epscol = np.full((P, 1), 1e-5, np.float32)
    colpack = np.zeros((P, 32), np.float32)
    for n, c0 in COL.items():
        key = {"bq_s": "bq_s", "bk_s": "bk_s", "bq_c": "bq_c",
               "bk_c": "bk_c", "bo_s": "bo_s", "bo_c": "bo_c",
               "b2": "b2"}[n]
        colpack[:, c0:c0 + 4] = g(key).reshape(4, P).T

    per_core_shared = {
        "wq_s": shared["wq_s"], "wk_s": shared["wk_s"],
        "wv_s": shared["wv_s"], "wq_c": shared["wq_c"],
        "wk_c": shared["wk_c"], "wv_c": shared["wv_c"],
        "wo_s": shared["wo_s"], "wo_c": shared["wo_c"],
        "w1": shared["w1"], "w1row": shared["w1row"], "w2": shared["w2"],
        "wq_s_row": shared["wq_s_row"][None, :],
        "wk_s_row": shared["wk_s_row"][None, :],
        "wv_s_row": shared["wv_s_row"][None, :],
        "wq_c_row": shared["wq_c_row"][None, :],
        "wk_c_row": shared["wk_c_row"][None, :],
        "wv_c_row": shared["wv_c_row"][None, :],
        "ones_rep": ones_rep, "wmink_rep": wmink, "sel8": sel8,
        "wminkcols": wminkcols, "selrep": selrep,
        "epscol": epscol, "colpack": colpack,
    }
    if bias_nz["bv_s"] or bias_nz["bv_c"]:
        bvrep = np.zeros((P, 2 * DS), np.float32)
        bvrep[:, 0:DS] = g("bv_s")[None, :]
        bvrep[:, DS:] = g("bv_c")[None, :]
        per_core_shared["bvrep"] = bvrep
    if bias_nz["b1"]:
        per_core_shared["b1col"] = _f32(g("b1").reshape(16, P).T)
    if any(gb_nz.values()):
        gbc = np.zeros((P, 24), np.float32)
        for i in (1, 2, 3):
            gbc[:, 8 * (i - 1):8 * (i - 1) + 4] = g("g%d" % i).reshape(4, P).T
            gbc[:, 8 * (i - 1) + 4:8 * (i - 1) + 8] = \
                g("beta%d" % i).reshape(4, P).T
        per_core_shared["gbcols"] = gbc
    if not src_all:
        srcm = np.asarray(inputs["src_mask"]).reshape(S)
        per_core_shared["srcb"] = _f32(
            np.where(srcm, 0.0, NEG).reshape(NSC, P).T)
    if tgt_kind == "general":
        per_core_shared["tmaskT"] = _f32(
            np.asarray(inputs["tgt_mask"]).reshape(T, S).T)

    x = g("x")
    enc = g("enc_output")
    in_maps = []
    for b in range(B):
        m = dict(per_core_shared)
        m["x_sp"] = _f32(x[b, :, 1:].T)
        m["x_t"] = _f32(np.broadcast_to(x[b, :, 0][None, :], (P, T)))
        m["e_sp"] = _f32(enc[b, :, 1:].T)
        rp = rowbase.copy()
        rp[0, 0:S] = enc[b, :, 0]
        m["rowpack"] = rp
        in_maps.append(m)
    return in_maps


def kernel(**inputs):
    import time as _time
    spec = _make_spec(inputs)
    nc = _NC_CACHE.get(spec)
    if nc is None:
        nc = _build(spec)
        _NC_CACHE[spec] = nc
    in_maps = _host_inputs(inputs, spec)
    res = None
    last_exc = None
    for attempt in range(3):
        try:
            res = bass_utils.run_bass_kernel_spmd(nc, in_maps,
                                                  core_ids=list(range(B)))
            break
        except Exception as e:  # transient device wedge: back off and retry
            last_exc = e
            _time.sleep(5.0)
    if res is None:
        raise last_exc
    out = np.empty((B, T, D), np.float32)
    for b in range(B):
        out[b, :, 1:] = res.results[b]["out_sp"].T
        out[b, :, 0] = res.results[b]["out_t"][0]
    return out

